# revision 1
# baseline (speedup 1.0000x reference)
"""Convpass adapter kernel for Trainium2, 8 NeuronCores, data-parallel over batch.

Computation (per image, N=1024 patches = 32x32 grid, C=768, dim=8):
    d1 = x @ Wd + bd                  # [N, 8]
    a1 = quick_gelu(d1)               # quick_gelu(v) = v*sigmoid(1.702v) = silu(1.702v)/1.702
    c2 = conv3x3(a1, Wc) + bc         # SAME padding on 32x32 grid
    a2 = quick_gelu(c2)
    out = a2 @ Wu + bu                # [N, 768]

Sharding: batch 64 -> 8 images per core. Host pre-transposes each core's x
shard to xT [768, 8192] so the C-contraction dim lands on SBUF partitions
(the down-projection contracts over C; a device-side transpose would cost
~100us of TensorE time, while the host layout change is free at HW-exec time).

Scaling trick: silu(1.702*(v+b)) = 1.702*quick_gelu(v+b), so each activation
is one ScalarE op (scale=1.702, bias=1.702*b, func=Silu); the 1.702 factors
are divided out of the downstream weights (Wc, Wu).

All matmul PSUM outputs start at partition 0 (ISA: dst col-group must begin
at group 0). The 3x3 conv batches 4 images as 9 PSUM-accumulated block-diagonal
[128x128] matmuls over a zero-padded [128, 34, 34] buffer (images at partition
strips 0/32/64/96; ScalarE handles the 32-aligned strip scatter/gather); the
padding ring is zeroed by DMA from a constant zero DRAM tensor. The up-projection
folds bu via a ones-row (K=9) so PSUM holds the final result, and PSUM->SBUF
copies alternate between VectorE and ScalarE.

Matmuls run in float32r (full-rate fp32 mode, ~1e-4 rel err); inputs are
declared float32r in DRAM so DMA delivers them pre-"rounded".
"""

import sys
import numpy as np

for _p in ("/opt/trn_rl_repo",):
    if _p not in sys.path:
        sys.path.append(_p)

import concourse.bacc as bacc
import concourse.mybir as mybir
import concourse.tile as tile
from concourse.bass_utils import run_bass_kernel_spmd

P = 128
N_CORES = 8
B, N, C, DIM = 64, 1024, 768, 8
IPC = B // N_CORES          # images per core
ROWS = IPC * N              # 8192
KC = C // P                 # 6 contraction chunks
H = 32                      # patch grid
AF = mybir.ActivationFunctionType
F32 = mybir.dt.float32
F32R = mybir.dt.float32r
GS = 1.702

_NC_CACHE = None


def _build_nc():
    nc = bacc.Bacc(None, target_bir_lowering=False)

    xT = nc.dram_tensor("xT", [KC, P, ROWS], F32R, kind="ExternalInput")
    wd = nc.dram_tensor("wd", [KC, P, DIM], F32R, kind="ExternalInput")
    wcbd = nc.dram_tensor("wcbd", [P, 9, P], F32R, kind="ExternalInput")
    wu3 = nc.dram_tensor("wu3", [DIM + 1, C], F32R, kind="ExternalInput")
    bdr = nc.dram_tensor("bdr", [DIM, 1], F32, kind="ExternalInput")
    bcr = nc.dram_tensor("bcr", [P, 1], F32, kind="ExternalInput")
    ones = nc.dram_tensor("ones", [1, N], F32R, kind="ExternalInput")
    zpad = nc.dram_tensor("zpad", [P, H + 2, H + 2], F32R, kind="ExternalInput")
    out = nc.dram_tensor("out", [ROWS, C], F32, kind="ExternalOutput")

    with tile.TileContext(nc) as tc:
        with (
            tc.tile_pool(name="const", bufs=1) as const,
            tc.tile_pool(name="xt", bufs=6) as xt_pool,
            tc.tile_pool(name="pad", bufs=2) as pad_pool,
            tc.tile_pool(name="s2", bufs=4) as s2_pool,
            tc.tile_pool(name="stag", bufs=4) as stag_pool,
            tc.tile_pool(name="ps_d", bufs=2, space="PSUM") as ps_d,
            tc.tile_pool(name="ps_c", bufs=2, space="PSUM") as ps_c,
            tc.tile_pool(name="ps_u", bufs=4, space="PSUM") as ps_u,
        ):
            prefetched = {}
            for n in range(2):
                xt = xt_pool.tile([P, KC, 512], F32R, name=f"xtpre{n}", tag="xt")
                nc.sync.dma_start(
                    xt[:],
                    xT[:, :, n * 512:(n + 1) * 512].rearrange("k p n -> p k n"),
                )
                prefetched[(0, n)] = xt
            wd_s = const.tile([P, KC, DIM], F32R)
            nc.sync.dma_start(wd_s[:], wd[:].rearrange("k p d -> p k d"))
            wcbd_s = const.tile([P, 9, P], F32R)
            nc.sync.dma_start(wcbd_s[:], wcbd[:])
            wu3_s = const.tile([DIM + 1, C], F32R)
            nc.sync.dma_start(wu3_s[:], wu3[:])
            bdr_s = const.tile([DIM, 1], F32)
            nc.sync.dma_start(bdr_s[:], bdr[:])
            bcr_s = const.tile([P, 1], F32)
            nc.sync.dma_start(bcr_s[:], bcr[:])

            for g in range(IPC // 4):
                padbuf = pad_pool.tile([P, H + 2, H + 2], F32R)
                nc.gpsimd.memset(padbuf[:].bitcast(F32), 0.0)

                for i in range(4):
                    img = 4 * g + i
                    for n in range(2):
                        xt = prefetched.pop((img, n), None)
                        if xt is None:
                            xt = xt_pool.tile([P, KC, 512], F32R, name="xt", tag="xt")
                            r0 = img * N + n * 512
                            nc.sync.dma_start(
                                xt[:],
                                xT[:, :, r0:r0 + 512].rearrange("k p n -> p k n"),
                            )
                        psd = ps_d.tile([DIM, 512], F32)
                        for k in range(KC):
                            nc.tensor.matmul(
                                psd[:],
                                wd_s[:, k, :],
                                xt[:, k, :],
                                start=(k == 0),
                                stop=(k == KC - 1),
                            )
                        # silu(1.702*(d1 + bd)) -> image strip of padded grid
                        nc.scalar.activation(
                            padbuf[32 * i:32 * i + DIM,
                                   1 + 16 * n:1 + 16 * n + 16, 1:33],
                            psd[:].rearrange("p (a b) -> p a b", a=16),
                            AF.Silu,
                            bias=bdr_s[:],
                            scale=GS,
                        )

                # 3x3 conv, 4 images at once: 9 block-diagonal matmuls per half
                pscs = []
                for n in range(2):
                    psc = ps_c.tile([P, 512], F32, tag="psc", name=f"psc{n}")
                    pscs.append(psc)
                    for t in range(9):
                        dy, dx = t // 3, t % 3
                        nc.tensor.matmul(
                            psc[:],
                            wcbd_s[:, t, :],
                            padbuf[:, 16 * n + dy:16 * n + dy + 16, dx:dx + 32],
                            start=(t == 0),
                            stop=(t == 8),
                        )

                for i in range(4):
                    img = 4 * g + i
                    s2g = s2_pool.tile([DIM + 1, N], F32R)
                    nc.sync.dma_start(s2g[DIM:DIM + 1, :], ones[:])
                    for n in range(2):
                        nc.scalar.activation(
                            s2g[0:DIM, n * 512:(n + 1) * 512],
                            pscs[n][32 * i:32 * i + DIM, :],
                            AF.Silu,
                            bias=bcr_s[32 * i:32 * i + DIM, :],
                            scale=GS,
                        )

                    # up-projection: out rows in chunks of 128, 512-row stores
                    for half in range(2):
                        stag = stag_pool.tile([P, 4, C], F32)
                        for a4 in range(4):
                            a = half * 4 + a4
                            for nn in range(2):
                                psu = ps_u.tile([P, 384], F32)
                                nc.tensor.matmul(
                                    psu[:],
                                    s2g[0:DIM + 1, a * P:(a + 1) * P],
                                    wu3_s[:, nn * 384:(nn + 1) * 384],
                                    start=True,
                                    stop=True,
                                )
                                dst = stag[:, a4, nn * 384:(nn + 1) * 384]
                                if nn == 0:
                                    nc.vector.tensor_copy(dst, psu[:])
                                else:
                                    nc.scalar.copy(dst, psu[:])
                        r0 = img * N + half * 512
                        nc.scalar.dma_start(
                            out[r0:r0 + 512, :].rearrange("(a p) c -> p a c", p=P),
                            stag[:],
                        )
    nc.compile()
    return nc


def _get_nc():
    global _NC_CACHE
    if _NC_CACHE is None:
        _NC_CACHE = _build_nc()
    return _NC_CACHE


def kernel(x, Wd, bd, Wc, bc, Wu, bu, _trace=False, _trace_kwargs=None):
    x = np.ascontiguousarray(x, dtype=np.float32)
    Wd = np.asarray(Wd, dtype=np.float32)
    bd = np.asarray(bd, dtype=np.float32)
    Wc = np.asarray(Wc, dtype=np.float32)
    bc = np.asarray(bc, dtype=np.float32)
    Wu = np.asarray(Wu, dtype=np.float32)
    bu = np.asarray(bu, dtype=np.float32)

    # shared (replicated) parameter prep
    wd_h = np.ascontiguousarray(Wd.reshape(KC, P, DIM))
    wcbd_h = np.zeros((P, 9, P), dtype=np.float32)
    for t in range(9):
        blk = (Wc[t // 3, t % 3] / GS)                       # [ci, co]
        for i in range(4):
            wcbd_h[32 * i:32 * i + DIM, t, 32 * i:32 * i + DIM] = blk
    wu3_h = np.concatenate([Wu / GS, bu[None, :]], axis=0)   # [9, 768]
    bdr_h = np.ascontiguousarray((GS * bd)[:, None])         # [8, 1]
    bcr_h = np.zeros((P, 1), dtype=np.float32)
    for i in range(4):
        bcr_h[32 * i:32 * i + DIM, 0] = GS * bc
    ones_h = np.ones((1, N), dtype=np.float32)
    zpad_h = np.zeros((P, H + 2, H + 2), dtype=np.float32)

    in_maps = []
    for c in range(N_CORES):
        shard = x[c * IPC:(c + 1) * IPC].reshape(ROWS, C)
        xT_h = np.ascontiguousarray(shard.T).reshape(KC, P, ROWS)
        in_maps.append({
            "xT": xT_h, "wd": wd_h, "wcbd": wcbd_h, "wu3": wu3_h,
            "bdr": bdr_h, "bcr": bcr_h, "ones": ones_h, "zpad": zpad_h,
        })

    nc = _get_nc()
    res = run_bass_kernel_spmd(
        nc, in_maps, core_ids=list(range(N_CORES)),
        trace=_trace, **(_trace_kwargs or {}),
    )
    kernel.last_result = res
    outs = [r["out"].reshape(IPC, N, C) for r in res.results]
    return np.concatenate(outs, axis=0)



# revision 2
# speedup vs baseline: 1.4093x; 1.4093x over previous
"""Convpass adapter kernel for Trainium2, 8 NeuronCores, data-parallel over batch.

Computation (per image, N=1024 patches = 32x32 grid, C=768, dim=8):
    d1 = x @ Wd + bd                  # [N, 8]
    a1 = quick_gelu(d1)               # quick_gelu(v) = v*sigmoid(1.702v) = silu(1.702v)/1.702
    c2 = conv3x3(a1, Wc) + bc         # SAME padding on 32x32 grid
    a2 = quick_gelu(c2)
    out = a2 @ Wu + bu                # [N, 768]

Sharding: batch 64 -> 8 images per core. The problem is HBM-bandwidth bound
(48 MiB/core of f32 I/O vs ~210 MFLOP of compute), so all device I/O is fp16:
the host quantizes x to fp16 (adds ~7e-4 rel err vs the 2e-2 budget) and
upcasts the fp16 output; HBM traffic halves to ~24.6 MiB/core.

Host-side layout prep (free at HW-exec time):
  - input  xt[p, i, k, n] = x[img i, patch n, chan 128k+p]: per-image loads are
    a single contiguous 12 KiB line per partition (1 descriptor/partition).
  - output out[i, k, p, n] = y[img i, patch n, chan 128k+p] (channel-major);
    the host transposes back. Stores are 6 x 2 KiB lines per partition.

Scaling trick: silu(1.702*(v+b)) = 1.702*quick_gelu(v+b), so each activation
is one ScalarE op (scale=1.702, bias=1.702*b, func=Silu); the 1.702 factors
are divided out of the downstream weights (Wc, Wu).

The 3x3 conv batches 4 images as 9 PSUM-accumulated block-diagonal [128x128]
matmuls over a zero-padded [128, 34, 34] fp16 buffer (images at partition
strips 0/32/64/96). The up-projection bias bu is folded via a ones-row with
K=9: row 8 of the activation output is produced by the conv activation itself
-- the conv weight rows for channel 8 of each strip are zero, so PSUM row
32i+8 is exactly 0, and its activation bias is v* with silu(v*)=1, making the
activation emit the constant 1.0 row for free (no extra DMA or instruction).

Up-projection runs stationary=wu3[9,128] / moving=s2g[9,512]: PSUM holds
[128 chans, 512 patches] and the PSUM->SBUF copies ([128,512] f32->fp16)
alternate between VectorE and ScalarE.
"""

import sys
import numpy as np

for _p in ("/opt/trn_rl_repo",):
    if _p not in sys.path:
        sys.path.append(_p)

import concourse.bacc as bacc
import concourse.mybir as mybir
import concourse.tile as tile
from concourse.bass_utils import run_bass_kernel_spmd

P = 128
N_CORES = 8
B, N, C, DIM = 64, 1024, 768, 8
IPC = B // N_CORES          # images per core
KC = C // P                 # 6 contraction chunks
H = 32                      # patch grid
AF = mybir.ActivationFunctionType
F32 = mybir.dt.float32
F16 = mybir.dt.float16
GS = 1.702
VSTAR = 1.2784645427610737  # silu(VSTAR) == 1.0

_NC_CACHE = None


def _build_nc():
    nc = bacc.Bacc(None, target_bir_lowering=False)

    xt_d = nc.dram_tensor("xt", [P, IPC, KC, N], F16, kind="ExternalInput")
    wd_d = nc.dram_tensor("wd", [P, KC, DIM], F16, kind="ExternalInput")
    wcbd_d = nc.dram_tensor("wcbd", [P, 9, P], F16, kind="ExternalInput")
    wu3_d = nc.dram_tensor("wu3", [DIM + 1, C], F16, kind="ExternalInput")
    bdr_d = nc.dram_tensor("bdr", [DIM, 1], F32, kind="ExternalInput")
    bcr_d = nc.dram_tensor("bcr", [P, 1], F32, kind="ExternalInput")
    out_d = nc.dram_tensor("out", [IPC, KC, P, N], F16, kind="ExternalOutput")

    with tile.TileContext(nc) as tc:
        with (
            tc.tile_pool(name="const", bufs=1) as const,
            tc.tile_pool(name="xt", bufs=4) as xt_pool,
            tc.tile_pool(name="pad", bufs=2) as pad_pool,
            tc.tile_pool(name="s2", bufs=4) as s2_pool,
            tc.tile_pool(name="stag", bufs=3) as stag_pool,
            tc.tile_pool(name="ps_d", bufs=2, space="PSUM") as ps_d,
            tc.tile_pool(name="ps_c", bufs=2, space="PSUM") as ps_c,
            tc.tile_pool(name="ps_u", bufs=4, space="PSUM") as ps_u,
        ):
            # consts ride the scalar HWDGE queue so the first image load (on
            # the sync queue) streams concurrently
            wd_s = const.tile([P, KC, DIM], F16)
            nc.scalar.dma_start(wd_s[:], wd_d[:])
            wcbd_s = const.tile([P, 9, P], F16)
            nc.scalar.dma_start(wcbd_s[:], wcbd_d[:])
            wu3_s = const.tile([DIM + 1, C], F16)
            nc.scalar.dma_start(wu3_s[:], wu3_d[:])
            bdr_s = const.tile([DIM, 1], F32)
            nc.scalar.dma_start(bdr_s[:], bdr_d[:])
            bcr_s = const.tile([P, 1], F32)
            nc.scalar.dma_start(bcr_s[:], bcr_d[:])

            xts = {}
            for i in range(2):
                t = xt_pool.tile([P, KC, N], F16, name=f"xtpre{i}", tag="xt")
                nc.sync.dma_start(t[:], xt_d[:, i])
                xts[i] = t

            for g in range(IPC // 4):
                padbuf = pad_pool.tile([P, H + 2, H + 2], F16)
                nc.gpsimd.memset(padbuf[:].bitcast(F32), 0.0)

                for i in range(4):
                    img = 4 * g + i
                    xt = xts.pop(img)
                    if img + 2 < IPC:
                        t = xt_pool.tile([P, KC, N], F16, name="xt", tag="xt")
                        nc.sync.dma_start(t[:], xt_d[:, img + 2])
                        xts[img + 2] = t
                    for h in range(2):
                        psd = ps_d.tile([DIM, 512], F32)
                        for k in range(KC):
                            nc.tensor.matmul(
                                psd[:],
                                wd_s[:, k, :],
                                xt[:, k, h * 512:(h + 1) * 512],
                                start=(k == 0),
                                stop=(k == KC - 1),
                            )
                        # silu(1.702*(d1 + bd)) -> image strip of padded grid
                        nc.scalar.activation(
                            padbuf[32 * i:32 * i + DIM,
                                   1 + 16 * h:1 + 16 * h + 16, 1:33],
                            psd[:].rearrange("p (a b) -> p a b", a=16),
                            AF.Silu,
                            bias=bdr_s[:],
                            scale=GS,
                        )

                # 3x3 conv, 4 images at once: 9 block-diagonal matmuls per half
                pscs = []
                for h in range(2):
                    psc = ps_c.tile([P, 512], F32, tag="psc", name=f"psc{h}")
                    pscs.append(psc)
                    for t9 in range(9):
                        dy, dx = t9 // 3, t9 % 3
                        nc.tensor.matmul(
                            psc[:],
                            wcbd_s[:, t9, :],
                            padbuf[:, 16 * h + dy:16 * h + dy + 16, dx:dx + 32],
                            start=(t9 == 0),
                            stop=(t9 == 8),
                        )

                for i in range(4):
                    img = 4 * g + i
                    # rows 0..7: silu-activated conv output; row 8: PSUM row is
                    # exactly 0 (zero conv-weight rows) and bias VSTAR makes the
                    # activation emit 1.0 -- the ones-row that folds bu into wu3
                    s2g = s2_pool.tile([DIM + 1, N], F16)
                    for h in range(2):
                        nc.scalar.activation(
                            s2g[0:DIM + 1, h * 512:(h + 1) * 512],
                            pscs[h][32 * i:32 * i + DIM + 1, :],
                            AF.Silu,
                            bias=bcr_s[32 * i:32 * i + DIM + 1, :],
                            scale=GS,
                        )

                    stag = stag_pool.tile([P, KC, N], F16)
                    for cc in range(KC):
                        for h in range(2):
                            psu = ps_u.tile([P, 512], F32)
                            nc.tensor.matmul(
                                psu[:],
                                wu3_s[:, cc * P:(cc + 1) * P],
                                s2g[0:DIM + 1, h * 512:(h + 1) * 512],
                                start=True,
                                stop=True,
                            )
                            dst = stag[:, cc, h * 512:(h + 1) * 512]
                            if (cc * 2 + h) % 2 == 0:
                                nc.vector.tensor_copy(dst, psu[:])
                            else:
                                nc.scalar.copy(dst, psu[:])
                    nc.scalar.dma_start(
                        out_d[img].rearrange("k p n -> p k n"), stag[:],
                    )
    nc.compile()
    return nc


def _get_nc():
    global _NC_CACHE
    if _NC_CACHE is None:
        _NC_CACHE = _build_nc()
    return _NC_CACHE


def kernel(x, Wd, bd, Wc, bc, Wu, bu, _trace=False, _trace_kwargs=None):
    x = np.asarray(x, dtype=np.float32)
    Wd = np.asarray(Wd, dtype=np.float32)
    bd = np.asarray(bd, dtype=np.float32)
    Wc = np.asarray(Wc, dtype=np.float32)
    bc = np.asarray(bc, dtype=np.float32)
    Wu = np.asarray(Wu, dtype=np.float32)
    bu = np.asarray(bu, dtype=np.float32)

    # shared (replicated) parameter prep
    wd_h = np.ascontiguousarray(
        Wd.astype(np.float16).reshape(KC, P, DIM).transpose(1, 0, 2))
    wcbd_h = np.zeros((P, 9, P), dtype=np.float16)
    for t9 in range(9):
        blk = (Wc[t9 // 3, t9 % 3] / GS).astype(np.float16)     # [ci, co]
        for i in range(4):
            wcbd_h[32 * i:32 * i + DIM, t9, 32 * i:32 * i + DIM] = blk
    wu3_h = np.concatenate(
        [Wu / GS, bu[None, :]], axis=0).astype(np.float16)       # [9, 768]
    bdr_h = np.ascontiguousarray((GS * bd)[:, None].astype(np.float32))
    bcr_h = np.zeros((P, 1), dtype=np.float32)
    for i in range(4):
        bcr_h[32 * i:32 * i + DIM, 0] = GS * bc
        bcr_h[32 * i + DIM, 0] = VSTAR

    x16 = x.astype(np.float16)                                   # [B, N, C]
    in_maps = []
    for c in range(N_CORES):
        sh = x16[c * IPC:(c + 1) * IPC]                          # [IPC, N, C]
        t = sh.transpose(2, 0, 1)                                # [C, IPC, N]
        xt_h = np.ascontiguousarray(
            t.reshape(KC, P, IPC, N).transpose(1, 2, 0, 3))      # [P,IPC,KC,N]
        in_maps.append({
            "xt": xt_h, "wd": wd_h, "wcbd": wcbd_h, "wu3": wu3_h,
            "bdr": bdr_h, "bcr": bcr_h,
        })

    nc = _get_nc()
    res = run_bass_kernel_spmd(
        nc, in_maps, core_ids=list(range(N_CORES)),
        trace=_trace, **(_trace_kwargs or {}),
    )
    kernel.last_result = res
    outs = []
    for r in res.results:
        o = r["out"].astype(np.float32)                          # [IPC,KC,P,N]
        outs.append(o.transpose(0, 3, 1, 2).reshape(IPC, N, C))
    return np.concatenate(outs, axis=0)


# revision 4
# speedup vs baseline: 1.5789x; 1.1203x over previous
"""Convpass adapter kernel for Trainium2, 8 NeuronCores, data-parallel over batch.

v2: fp16 I/O + PE row-tiling for the up-projection.

Computation (per image, N=1024 patches = 32x32 grid, C=768, dim=8):
    d1 = x @ Wd + bd                  # [N, 8]
    a1 = quick_gelu(d1)               # quick_gelu(v) = v*sigmoid(1.702v) = silu(1.702v)/1.702
    c2 = conv3x3(a1, Wc) + bc         # SAME padding on 32x32 grid
    a2 = quick_gelu(c2)
    out = a2 @ Wu + bu                # [N, 768]

Sharding: batch 64 -> 8 images per core. The problem is HBM-bandwidth bound,
so all device I/O is fp16 (host quantizes x / upcasts out; ~7e-4 rel err vs
the 2e-2 budget); HBM traffic halves to ~24.6 MiB/core.

Host-side layout prep (free at HW-exec time):
  - input  xt[p, i, k, n] = x[img i, patch n, chan 128k+p]: per-image loads are
    a single contiguous 12 KiB line per partition.
  - output out[i, k, p, n] = y[img i, patch n, chan 128k+p] (channel-major).

The 3x3 conv batches 4 images as 9 PSUM-accumulated block-diagonal [128x128]
matmuls over a zero-padded [128, 34, 34] fp16 buffer (images at partition
strips 0/32/64/96). The conv activation emits the whole 128-partition half in
ONE ScalarE op into a strip-stacked s2g buffer; strip row 32i+8 is the
ones-row for folding bu (conv-weight rows there are zero so PSUM is exactly 0,
and its activation bias is v* with silu(v*)=1).

Up-projection uses PE row tiling: contraction K=9 rounds to a 32-row tile, so
the 4 images' matmuls (stationary wu3 replicated at partition strips 0/32/64/
96, moving s2g strips in place) land on row groups 0/32/64/96 and execute
CONCURRENTLY in the PE array (~4x effective throughput for this phase, which
keeps TensorE under the DMA roofline even at the low HAM p-state).

Scaling trick: silu(1.702*(v+b)) = 1.702*quick_gelu(v+b), so each activation
is one ScalarE op (scale=1.702, bias=1.702*b, func=Silu); the 1.702 factors
are divided out of the downstream weights (Wc, Wu).
"""

import sys
import numpy as np

for _p in ("/opt/trn_rl_repo",):
    if _p not in sys.path:
        sys.path.append(_p)

import concourse.bacc as bacc
import concourse.mybir as mybir
import concourse.tile as tile
from concourse.bass_utils import run_bass_kernel_spmd

P = 128
N_CORES = 8
B, N, C, DIM = 64, 1024, 768, 8
IPC = B // N_CORES          # images per core
KC = C // P                 # 6 contraction chunks
H = 32                      # patch grid
AF = mybir.ActivationFunctionType
F32 = mybir.dt.float32
F16 = mybir.dt.float16
GS = 1.702
VSTAR = 1.2784645427610737  # silu(VSTAR) == 1.0

_NC_CACHE = None


def _build_nc():
    nc = bacc.Bacc(None, target_bir_lowering=False)

    xt_d = nc.dram_tensor("xt", [P, IPC, KC, N], F16, kind="ExternalInput")
    wd_d = nc.dram_tensor("wd", [P, KC, DIM], F16, kind="ExternalInput")
    wcbd_d = nc.dram_tensor("wcbd", [P, 9, P], F16, kind="ExternalInput")
    wu3r_d = nc.dram_tensor("wu3r", [P, C], F16, kind="ExternalInput")
    bdr_d = nc.dram_tensor("bdr", [DIM, 1], F32, kind="ExternalInput")
    bcr_d = nc.dram_tensor("bcr", [P, 1], F32, kind="ExternalInput")
    out_d = nc.dram_tensor("out", [IPC, KC, P, N], F16, kind="ExternalOutput")

    with tile.TileContext(nc) as tc:
        with (
            tc.tile_pool(name="const", bufs=1) as const,
            tc.tile_pool(name="xt", bufs=4) as xt_pool,
            tc.tile_pool(name="pad", bufs=2) as pad_pool,
            tc.tile_pool(name="s2", bufs=4) as s2_pool,
            tc.tile_pool(name="stag", bufs=8) as stag_pool,
            tc.tile_pool(name="ps_d", bufs=2, space="PSUM") as ps_d,
            tc.tile_pool(name="ps_c", bufs=2, space="PSUM") as ps_c,
            tc.tile_pool(name="ps_u", bufs=4, space="PSUM") as ps_u,
        ):
            # consts ride the scalar HWDGE queue so the first image load (on
            # the sync queue) streams concurrently
            wd_s = const.tile([P, KC, DIM], F16)
            nc.scalar.dma_start(wd_s[:], wd_d[:])
            wcbd_s = const.tile([P, 9, P], F16)
            nc.scalar.dma_start(wcbd_s[:], wcbd_d[:])
            wu3r_s = const.tile([P, C], F16)
            nc.scalar.dma_start(wu3r_s[:], wu3r_d[:])
            bdr_s = const.tile([DIM, 1], F32)
            nc.scalar.dma_start(bdr_s[:], bdr_d[:])
            bcr_s = const.tile([P, 1], F32)
            nc.scalar.dma_start(bcr_s[:], bcr_d[:])

            xts = {}
            for i in range(2):
                t = xt_pool.tile([P, KC, N], F16, name=f"xtpre{i}", tag="xt")
                nc.sync.dma_start(t[:], xt_d[:, i])
                xts[i] = t

            for g in range(IPC // 4):
                padbuf = pad_pool.tile([P, H + 2, H + 2], F16)
                nc.gpsimd.memset(padbuf[:].bitcast(F32), 0.0)

                for i in range(4):
                    img = 4 * g + i
                    xt = xts.pop(img)
                    if img + 2 < IPC:
                        t = xt_pool.tile([P, KC, N], F16, name="xt", tag="xt")
                        nc.sync.dma_start(t[:], xt_d[:, img + 2])
                        xts[img + 2] = t
                    for h in range(2):
                        psd = ps_d.tile([DIM, 512], F32)
                        for k in range(KC):
                            nc.tensor.matmul(
                                psd[:],
                                wd_s[:, k, :],
                                xt[:, k, h * 512:(h + 1) * 512],
                                start=(k == 0),
                                stop=(k == KC - 1),
                            )
                        # silu(1.702*(d1 + bd)) -> image strip of padded grid
                        nc.scalar.activation(
                            padbuf[32 * i:32 * i + DIM,
                                   1 + 16 * h:1 + 16 * h + 16, 1:33],
                            psd[:].rearrange("p (a b) -> p a b", a=16),
                            AF.Silu,
                            bias=bdr_s[:],
                            scale=GS,
                        )

                # 3x3 conv, 4 images at once: 9 block-diagonal matmuls per half
                s2gs = []
                for h in range(2):
                    psc = ps_c.tile([P, 512], F32, tag="psc", name=f"psc{h}")
                    for t9 in range(9):
                        dy, dx = t9 // 3, t9 % 3
                        nc.tensor.matmul(
                            psc[:],
                            wcbd_s[:, t9, :],
                            padbuf[:, 16 * h + dy:16 * h + dy + 16, dx:dx + 32],
                            start=(t9 == 0),
                            stop=(t9 == 8),
                        )
                    # one activation for all 4 strips; strip row 32i+8 becomes
                    # the ones-row (PSUM there is exactly 0, silu(VSTAR)=1)
                    s2g = s2_pool.tile([P, 512], F16, tag="s2g", name=f"s2g{h}")
                    s2gs.append(s2g)
                    nc.scalar.activation(
                        s2g[:],
                        psc[:],
                        AF.Silu,
                        bias=bcr_s[:],
                        scale=GS,
                    )

                # up-projection: row-tiled quads -- the 4 images' matmuls use
                # disjoint 32-row PE groups and run concurrently
                stags = [stag_pool.tile([P, KC, N], F16, name="stag", tag="stag")
                         for _ in range(4)]
                for cc in range(KC):
                    for h in range(2):
                        for i in range(4):
                            psu = ps_u.tile([P, 512], F32)
                            # explicit tile_position: the auto-infer path
                            # rejects base partition 96
                            nc.tensor.matmul(
                                psu[:],
                                wu3r_s[32 * i:32 * i + DIM + 1,
                                       cc * P:(cc + 1) * P],
                                s2gs[h][32 * i:32 * i + DIM + 1, :],
                                start=True,
                                stop=True,
                                tile_position=(32 * i, 0),
                            )
                            dst = stags[i][:, cc, h * 512:(h + 1) * 512]
                            if (cc * 2 + h + i) % 2 == 0:
                                nc.vector.tensor_copy(dst, psu[:])
                            else:
                                nc.scalar.copy(dst, psu[:])
                for i in range(4):
                    img = 4 * g + i
                    # stores ride the gpsimd SWDGE ring: keeps the scalar
                    # engine free for activations/copies and gives stores
                    # their own descriptor stream alongside the sync loads
                    nc.gpsimd.dma_start(
                        out_d[img].rearrange("k p n -> p k n"), stags[i][:],
                    )
    nc.compile()
    return nc


def _get_nc():
    global _NC_CACHE
    if _NC_CACHE is None:
        _NC_CACHE = _build_nc()
    return _NC_CACHE


def kernel(x, Wd, bd, Wc, bc, Wu, bu, _trace=False, _trace_kwargs=None):
    x = np.asarray(x, dtype=np.float32)
    Wd = np.asarray(Wd, dtype=np.float32)
    bd = np.asarray(bd, dtype=np.float32)
    Wc = np.asarray(Wc, dtype=np.float32)
    bc = np.asarray(bc, dtype=np.float32)
    Wu = np.asarray(Wu, dtype=np.float32)
    bu = np.asarray(bu, dtype=np.float32)

    # shared (replicated) parameter prep
    wd_h = np.ascontiguousarray(
        Wd.astype(np.float16).reshape(KC, P, DIM).transpose(1, 0, 2))
    wcbd_h = np.zeros((P, 9, P), dtype=np.float16)
    for t9 in range(9):
        blk = (Wc[t9 // 3, t9 % 3] / GS).astype(np.float16)     # [ci, co]
        for i in range(4):
            wcbd_h[32 * i:32 * i + DIM, t9, 32 * i:32 * i + DIM] = blk
    wu3_h = np.concatenate(
        [Wu / GS, bu[None, :]], axis=0).astype(np.float16)       # [9, 768]
    wu3r_h = np.zeros((P, C), dtype=np.float16)
    for i in range(4):
        wu3r_h[32 * i:32 * i + DIM + 1] = wu3_h
    bdr_h = np.ascontiguousarray((GS * bd)[:, None].astype(np.float32))
    bcr_h = np.zeros((P, 1), dtype=np.float32)
    for i in range(4):
        bcr_h[32 * i:32 * i + DIM, 0] = GS * bc
        bcr_h[32 * i + DIM, 0] = VSTAR

    x16 = x.astype(np.float16)                                   # [B, N, C]
    in_maps = []
    for c in range(N_CORES):
        sh = x16[c * IPC:(c + 1) * IPC]                          # [IPC, N, C]
        t = sh.transpose(2, 0, 1)                                # [C, IPC, N]
        xt_h = np.ascontiguousarray(
            t.reshape(KC, P, IPC, N).transpose(1, 2, 0, 3))      # [P,IPC,KC,N]
        in_maps.append({
            "xt": xt_h, "wd": wd_h, "wcbd": wcbd_h, "wu3r": wu3r_h,
            "bdr": bdr_h, "bcr": bcr_h,
        })

    nc = _get_nc()
    res = run_bass_kernel_spmd(
        nc, in_maps, core_ids=list(range(N_CORES)),
        trace=_trace, **(_trace_kwargs or {}),
    )
    kernel.last_result = res
    outs = []
    for r in res.results:
        o = r["out"].astype(np.float32)                          # [IPC,KC,P,N]
        outs.append(o.transpose(0, 3, 1, 2).reshape(IPC, N, C))
    return np.concatenate(outs, axis=0)
